# revision 1
# baseline (speedup 1.0000x reference)
"""Causal self-attention with RoPE on 8 TRN2 NeuronCores.

Head-parallel tensor parallelism: core i owns heads 2i, 2i+1. Each core
computes its slice of the qkv projection, per-head causal attention
entirely in SBUF, and a partial output projection over its 128 channels;
a column-chunked ReduceScatter sums partials and leaves each core with
its 512-row shard of the output.

All matmuls run in float32r (full PE rate, ~tf32 mantissa). Erratum
rules respected: no f32r transpose / explicit tile_position, no mixed
base partitions inside one PSUM accumulation group.
"""

import numpy as np

import concourse.bass as bass
import concourse.mybir as mybir
import concourse.tile as tile
from concourse import bacc
from concourse.bass_utils import run_bass_kernel_spmd

F32 = mybir.dt.float32
F32R = mybir.dt.float32r
BF16 = mybir.dt.bfloat16

B, T, C = 2, 2048, 1024
H, HD = 16, 64
NC = 8
HL = H // NC          # heads per core = 2
BT = B * T            # 4096
FQKV = 3 * HL * HD    # 384 rows of w_attn per core
TSH = BT // NC        # 512 output rows per core
NCH = BT // 512       # 8 column chunks of the [*, BT] activations
ROPE_BASE = 10000.0


def build():
    nc = bacc.Bacc(None, target_bir_lowering=False)

    xT_d = nc.dram_tensor("xT", [C, BT], F32R, kind="ExternalInput")
    wq_d = nc.dram_tensor("wqkvT", [C, FQKV], F32R, kind="ExternalInput")
    wp_d = nc.dram_tensor("wpT", [128, C], F32R, kind="ExternalInput")
    cos_d = nc.dram_tensor("cosT", [128, BT], F32R, kind="ExternalInput")
    sin_d = nc.dram_tensor("sinT", [128, BT], F32R, kind="ExternalInput")
    perm_d = nc.dram_tensor("permT", [128, 128], F32R, kind="ExternalInput")
    mask_d = nc.dram_tensor("masks", [4, 128, 512], F32R, kind="ExternalInput")
    id_d = nc.dram_tensor("ident", [128, 128], F32, kind="ExternalInput")
    out_d = nc.dram_tensor("out", [TSH, C], F32, kind="ExternalOutput")

    # chunk (b, j): batch b, columns [512j, 512j+512), bf16 to halve RS bytes
    partial_c = {b_: nc.dram_tensor(f"partial{b_}", [T, C], BF16)
                 for b_ in range(2)}
    den_dram = {(b_, h_, qc_): nc.dram_tensor(f"dend{b_}{h_}{qc_}", [1, 512],
                                              F32)
                for b_ in range(2) for h_ in range(2) for qc_ in range(4)}
    rs_c = {b_: nc.dram_tensor(f"rs{b_}", [T // NC, C], BF16)
            for b_ in range(2)}

    with tile.TileContext(nc) as tc:
        with (
            tc.tile_pool(name="persist", bufs=1) as pp,
            tc.tile_pool(name="work", bufs=2) as wk,
            tc.tile_pool(name="pts", bufs=10) as ptp,
            tc.tile_pool(name="psA", bufs=2, space="PSUM") as psA,
            tc.tile_pool(name="psS", bufs=2, space="PSUM") as psS,
            tc.tile_pool(name="psV", bufs=4, space="PSUM") as psV,
        ):
            # ---- constants / weights (persist) ----
            wq_sb = []
            for c in range(8):
                t = pp.tile([128, FQKV], F32R, name=f"wq{c}", tag=f"wq{c}")
                nc.sync.dma_start(t[:], wq_d[c * 128:(c + 1) * 128, :])
                wq_sb.append(t)
            wp_sb = pp.tile([128, C], F32R, name="wp_sb", tag="wp_sb")
            nc.gpsimd.dma_start(wp_sb[:], wp_d[:])
            perm_sb = pp.tile([128, 128], F32R, name="perm_sb", tag="perm_sb")
            nc.gpsimd.dma_start(perm_sb[:], perm_d[:])
            id_sb = pp.tile([128, 128], F32, name="id_sb", tag="id_sb")
            nc.gpsimd.dma_start(id_sb[:], id_d[:])
            mask_sb = []
            for m in range(4):
                t = pp.tile([128, 512], F32R, name=f"mask{m}", tag=f"mask{m}")
                nc.gpsimd.dma_start(t[:], mask_d[m])
                mask_sb.append(t)
            ones_c = pp.tile([128, 1], F32, name="ones_c", tag="ones_c")
            nc.vector.memset(ones_c[:], 1.0)
            ones_r = pp.tile([1, HD], F32R, name="ones_r", tag="ones_r")
            nc.vector.tensor_copy(ones_r[:], ones_c[0:1, 0:1].broadcast_to((1, HD)))

            # chunked activations: 8 chunks of [128, 512] each
            qtc = [pp.tile([128, 512], F32R, name=f"qtc{i}", tag=f"qtc{i}")
                   for i in range(NCH)]
            ktc = [pp.tile([128, 512], F32R, name=f"ktc{i}", tag=f"ktc{i}")
                   for i in range(NCH)]
            vtc = [pp.tile([128, 512], F32, name=f"vtc{i}", tag=f"vtc{i}")
                   for i in range(NCH)]
            fdst = [qtc, ktc, vtc]

            # ---- phase 1: qkvT = wqkvT.T @ xT, [f, t] layout ----
            for th in range(4):          # t quarters to bound xT residency
                xt_sb = []
                for c in range(8):
                    t = pp.tile([128, 1024], F32R, name=f"xt{th}{c}",
                                tag=f"xt{c}")
                    nc.sync.dma_start(t[:], xT_d[c * 128:(c + 1) * 128,
                                                 th * 1024:(th + 1) * 1024])
                    xt_sb.append(t)
                for f in range(3):
                    pq = [psA.tile([128, 512], F32, name=f"pq{th}{f}{tq}",
                                   tag="ps_a") for tq in range(2)]
                    for c in range(8):
                        for tq in range(2):
                            nc.tensor.matmul(
                                pq[tq][:],
                                wq_sb[c][:, f * 128:(f + 1) * 128],
                                xt_sb[c][:, tq * 512:(tq + 1) * 512],
                                start=(c == 0), stop=(c == 7),
                            )
                    for tq in range(2):
                        nc.scalar.copy(fdst[f][th * 2 + tq][:], pq[tq][:])

            # ---- phase 2: RoPE on q, k chunks (in place) ----
            for ch in range(NCH):
                cosc = wk.tile([128, 512], F32R, name=f"cosc{ch}", tag="cosc")
                nc.gpsimd.dma_start(cosc[:], cos_d[:, ch * 512:(ch + 1) * 512])
                sinc = wk.tile([128, 512], F32R, name=f"sinc{ch}", tag="sinc")
                nc.gpsimd.dma_start(sinc[:], sin_d[:, ch * 512:(ch + 1) * 512])
                for which, tcl in (("q", qtc), ("k", ktc)):
                    src = tcl[ch]
                    pr = psA.tile([128, 512], F32, name=f"pr{which}{ch}",
                                  tag="ps_a")
                    nc.tensor.matmul(pr[:], perm_sb[:], src[:],
                                     start=True, stop=True)
                    rot = wk.tile([128, 512], F32R, name=f"rot{which}{ch}",
                                  tag="rot")
                    nc.vector.tensor_mul(rot[:], pr[:], sinc[:])
                    nc.vector.tensor_mul(src[:], src[:], cosc[:])
                    nc.vector.tensor_add(src[:], src[:], rot[:])

            # ---- phase 3: V blocks [t, d] with ones columns ----
            v_sb = []
            for kb in range(BT // 128):  # 32 key blocks across both batches
                pv = psA.tile([128, 128], F32, name=f"pv{kb}", tag="ps_a")
                nc.tensor.transpose(
                    pv[:], vtc[kb // 4][:, (kb % 4) * 128:(kb % 4 + 1) * 128],
                    id_sb[:])
                v = pp.tile([128, 2 * (HD + 1)], F32R, name=f"v{kb}",
                            tag=f"v{kb}")
                nc.vector.tensor_copy(v[:, 0:HD], pv[:, 0:HD])
                nc.vector.tensor_copy(v[:, HD + 1:2 * HD + 1],
                                      pv[:, HD:2 * HD])
                nc.vector.tensor_copy(v[:, HD:HD + 1], ones_c[:])
                nc.vector.tensor_copy(v[:, 2 * HD + 1:2 * HD + 2], ones_c[:])
                v_sb.append(v)

            # ---- phase 4: attention per (batch, head), kb-outer ----
            # combined per-chunk attention tiles: h0 rows 0:64, h1 rows 64:128
            atc = [pp.tile([128, 512], F32R, name=f"atc{i}", tag=f"atc{i}")
                   for i in range(NCH)]

            def normalize(b, h, qc, avq):
                den = wk.tile([HD + 1, 512], F32, name=f"den{b}{h}{qc}",
                              tag="den")
                nc.scalar.copy(den[HD:HD + 1, :], avq[HD:HD + 1, :])
                nc.sync.dma_start(den_dram[b, h, qc][:], den[HD:HD + 1, :])
                bcd = wk.tile([HD, 512], F32, name=f"bcd{b}{h}{qc}",
                              tag="bcd")
                nc.sync.dma_start(
                    bcd[:], den_dram[b, h, qc][0:1, :].broadcast_to((HD, 512)))
                bc = wk.tile([HD, 512], F32, name=f"bc{b}{h}{qc}", tag="bc")
                scr = wk.tile([HD, 512], F32, name=f"scr{b}{h}{qc}",
                              tag="scr")
                nc.vector.reciprocal_approx_accurate(
                    out=bc[:], in_=bcd[:], scratch=scr[:])
                if h == 0:
                    nc.vector.tensor_mul(atc[b * 4 + qc][0:HD, :],
                                         avq[0:HD, :], bc[:])
                else:
                    ath1 = wk.tile([HD, 512], F32R, name=f"ath1{b}{qc}",
                                   tag="ath1")
                    nc.vector.tensor_mul(ath1[:], avq[0:HD, :], bc[:])
                    nc.sync.dma_start(atc[b * 4 + qc][HD:128, :], ath1[:])

            def attention(b, on_qc_done):
                for h in range(HL):
                    hp = h * 64
                    avp = [psV.tile([HD + 1, 512], F32, name=f"av{b}{h}{q_}",
                                    tag="ps_av") for q_ in range(4)]

                    def s_group(kb, b=b, h=h, hp=hp):
                        """S^T + exp (+mask) for all valid q chunks of kb."""
                        res = []
                        kch = ktc[b * 4 + kb // 4]
                        koff = (kb % 4) * 128
                        for qc in range(kb // 4, 4):
                            sps = psS.tile([128, 512], F32,
                                           name=f"s{b}{h}{kb}{qc}",
                                           tag="ps_s")
                            nc.tensor.matmul(
                                sps[:],
                                kch[hp:hp + 64, koff:koff + 128],
                                qtc[b * 4 + qc][hp:hp + 64, :],
                                start=True, stop=True,
                            )
                            pt = ptp.tile([128, 512], F32R,
                                          name=f"pt{b}{h}{kb}{qc}", tag="pt")
                            nc.scalar.activation(
                                pt[:], sps[:],
                                mybir.ActivationFunctionType.Exp,
                                scale=0.125,
                            )
                            if qc == kb // 4:
                                nc.vector.tensor_mul(
                                    pt[:], pt[:], mask_sb[kb % 4][:])
                            res.append((qc, pt))
                        return res

                    def av_group(kb, pts, b=b, h=h, avp=avp):
                        for qc, pt in pts:
                            nc.tensor.matmul(
                                avp[qc][:],
                                v_sb[b * 16 + kb][:, h * (HD + 1):
                                                  (h + 1) * (HD + 1)],
                                pt[:],
                                start=(kb == 0), stop=(kb == 4 * qc + 3),
                            )

                    # software-pipeline S one kb-group ahead of AV; emit the
                    # normalize chain for q chunk qc right after its AV stop
                    # (kb == 4qc+3) so downstream engines start early.
                    prev = s_group(0)
                    for kb in range(1, 16):
                        cur = s_group(kb)
                        av_group(kb - 1, prev)
                        if (kb - 1) % 4 == 3:
                            qc = (kb - 1) // 4
                            normalize(b, h, qc, avp[qc])
                            if h == 1:
                                on_qc_done(b, qc)
                        prev = cur
                    av_group(15, prev)
                    normalize(b, h, 3, avp[3])
                    if h == 1:
                        on_qc_done(b, 3)

            # ---- phase 5: partial out-proj per (batch, col-half) + RS ----
            def outproj_group(b, qc):
                """Projection for the 4 t-blocks of chunk (b, qc), both
                column halves."""
                for j in range(2):
                    osl = slice(j * 512, (j + 1) * 512)
                    for sub in range(4):
                        tb16 = qc * 4 + sub
                        tb = b * 16 + tb16
                        po = psA.tile([128, 512], F32,
                                      name=f"po{b}{j}{tb16}", tag="ps_a")
                        nc.tensor.matmul(
                            po[:],
                            atc[tb // 4][:, (tb % 4) * 128:
                                         (tb % 4 + 1) * 128],
                            wp_sb[:, osl],
                            start=True, stop=True,
                        )
                        st = wk.tile([128, 512], BF16,
                                     name=f"st{b}{j}{tb16}", tag="st")
                        nc.vector.tensor_copy(st[:], po[:])
                        nc.sync.dma_start(
                            partial_c[b][tb16 * 128:(tb16 + 1) * 128, osl],
                            st[:])

            def rs_issue(b):
                nc.gpsimd.collective_compute(
                    "ReduceScatter",
                    mybir.AluOpType.add,
                    replica_groups=[list(range(NC))],
                    ins=[partial_c[b][:]],
                    outs=[rs_c[b][:]],
                )

            def rs_out(b):
                for r in range(T // NC // 128):
                    rsb = wk.tile([128, C], BF16, name=f"rsb{b}{r}",
                                  tag="rsb")
                    nc.sync.dma_start(
                        rsb[:], rs_c[b][r * 128:(r + 1) * 128, :])
                    rsf = wk.tile([128, C], F32, name=f"rsf{b}{r}",
                                  tag="rsf")
                    nc.vector.tensor_copy(rsf[:], rsb[:])
                    nc.sync.dma_start(
                        out_d[b * (T // NC) + r * 128:
                              b * (T // NC) + (r + 1) * 128, :], rsf[:])

            attention(0, outproj_group)
            rs_issue(0)
            attention(1, outproj_group)
            rs_issue(1)
            rs_out(0)
            rs_out(1)

    nc.finalize()
    return nc


def host_inputs(x, w_attn, w_proj):
    """Host-side sharding/layout prep. Returns per-core in_maps."""
    x2 = np.ascontiguousarray(x.reshape(BT, C).T).astype(np.float32)  # [C,BT]

    inv = 1.0 / (ROPE_BASE ** (np.arange(0, HD, 2, dtype=np.float32) / HD))
    tpos = np.arange(T, dtype=np.float32)
    freqs = tpos[:, None] * inv[None, :]                  # [T, 32]
    emb = np.concatenate([freqs, freqs], axis=-1)         # [T, 64]
    cosT = np.cos(emb).T.astype(np.float32)               # [64, T]
    sinT = np.sin(emb).T.astype(np.float32)
    cos_full = np.ascontiguousarray(np.tile(cosT, (2, B)))  # [128, BT]
    sin_full = np.ascontiguousarray(np.tile(sinT, (2, B)))

    m64 = np.zeros((HD, HD), dtype=np.float32)
    half = HD // 2
    for d in range(half):
        m64[d, d + half] = -1.0
        m64[d + half, d] = 1.0
    perm = np.zeros((128, 128), dtype=np.float32)
    perm[0:HD, 0:HD] = m64
    perm[HD:128, HD:128] = m64
    permT = np.ascontiguousarray(perm.T)

    masks = np.zeros((4, 128, 512), dtype=np.float32)
    qi = np.arange(512)[None, :]
    ki = np.arange(128)[:, None]
    for m in range(4):
        masks[m] = (qi - ki >= m * 128).astype(np.float32)

    ident = np.eye(128, dtype=np.float32)

    in_maps = []
    for i in range(NC):
        r0 = i * (HL * HD)
        wq = w_attn[r0:r0 + HL * HD, :]
        wk_ = w_attn[C + r0:C + r0 + HL * HD, :]
        wv = w_attn[2 * C + r0:2 * C + r0 + HL * HD, :]
        wqkvT = np.ascontiguousarray(
            np.concatenate([wq, wk_, wv], axis=0).T).astype(np.float32)
        c0 = i * (HL * HD)
        wpT = np.ascontiguousarray(
            w_proj[:, c0:c0 + 2 * HD].T).astype(np.float32)
        in_maps.append({
            "xT": x2, "wqkvT": wqkvT, "wpT": wpT,
            "cosT": cos_full, "sinT": sin_full, "permT": permT,
            "masks": masks, "ident": ident,
        })
    return in_maps


_NC_CACHE = None


def _get_nc():
    global _NC_CACHE
    if _NC_CACHE is None:
        _NC_CACHE = build()
    return _NC_CACHE


def run(x, w_attn, w_proj, trace=False):
    nc = _get_nc()
    in_maps = host_inputs(np.asarray(x), np.asarray(w_attn),
                          np.asarray(w_proj))
    res = run_bass_kernel_spmd(nc, in_maps, list(range(NC)), trace=trace)
    # core i returns [512, 1024]: rows 0:256 = batch0 rows [256i, 256i+256),
    # rows 256:512 = batch1 rows [256i, 256i+256)
    out = np.empty((B, T, C), dtype=np.float32)
    piece = T // NC
    for i in range(NC):
        sh = res.results[i]["out"]
        out[0, i * piece:(i + 1) * piece] = sh[0:piece]
        out[1, i * piece:(i + 1) * piece] = sh[piece:2 * piece]
    return out, res


def kernel(x, w_attn, w_proj):
    out, _ = run(x, w_attn, w_proj, trace=False)
    return out



# revision 8
# speedup vs baseline: 1.2423x; 1.2423x over previous
"""Causal self-attention with RoPE on 8 TRN2 NeuronCores.

Head-parallel tensor parallelism: core i owns heads 2i, 2i+1. Each core
computes its slice of the qkv projection, per-head causal attention
entirely in SBUF, and a partial output projection over its 128 channels;
a column-chunked ReduceScatter sums partials and leaves each core with
its 512-row shard of the output.

All matmuls run in float32r (full PE rate, ~tf32 mantissa). Erratum
rules respected: no f32r transpose / explicit tile_position, no mixed
base partitions inside one PSUM accumulation group.
"""

import ml_dtypes
import numpy as np

import concourse.bass as bass
import concourse.mybir as mybir
import concourse.tile as tile
from concourse import bacc
from concourse.bass_utils import run_bass_kernel_spmd

F32 = mybir.dt.float32
F32R = mybir.dt.float32r
BF16 = mybir.dt.bfloat16

B, T, C = 2, 2048, 1024
H, HD = 16, 64
NC = 8
HL = H // NC          # heads per core = 2
BT = B * T            # 4096
FQKV = 3 * HL * HD    # 384 rows of w_attn per core
TSH = BT // NC        # 512 output rows per core
NCH = BT // 512       # 8 column chunks of the [*, BT] activations
ROPE_BASE = 10000.0


def build():
    nc = bacc.Bacc(None, target_bir_lowering=False)

    xT_d = nc.dram_tensor("xT", [C, BT], F32R, kind="ExternalInput")
    wq_d = nc.dram_tensor("wqkvT", [C, FQKV], F32R, kind="ExternalInput")
    wp_d = nc.dram_tensor("wpT", [C, C], BF16, kind="ExternalInput")
    cos_d = nc.dram_tensor("cosT", [128, BT], F32R, kind="ExternalInput")
    sin_d = nc.dram_tensor("sinT", [128, BT], F32R, kind="ExternalInput")
    perm_d = nc.dram_tensor("permT", [128, 128], F32R, kind="ExternalInput")
    mask_d = nc.dram_tensor("masks", [4, 128, 512], F32R, kind="ExternalInput")
    id_d = nc.dram_tensor("ident", [128, 128], F32, kind="ExternalInput")
    out_d = nc.dram_tensor("out", [TSH, C], F32, kind="ExternalOutput")

    den_dram = {(b_, h_, qc_): nc.dram_tensor(f"dend{b_}{h_}{qc_}", [1, 512],
                                              F32)
                for b_ in range(2) for h_ in range(2) for qc_ in range(4)}
    # AllToAll staging: block j (rows 128j:128j+128) = this core's 128
    # attention-output channels for core j's 256 tokens of batch b.
    a2a_in = {b_: nc.dram_tensor(f"a2ain{b_}", [NC * 128, T // NC], BF16)
              for b_ in range(2)}
    a2a_out = {b_: nc.dram_tensor(f"a2aout{b_}", [NC * 128, T // NC], BF16)
               for b_ in range(2)}

    with tile.TileContext(nc) as tc:
        with (
            tc.tile_pool(name="persist", bufs=1) as pp,
            tc.tile_pool(name="work", bufs=2) as wk,
            tc.tile_pool(name="pts", bufs=10) as ptp,
            tc.tile_pool(name="psA", bufs=2, space="PSUM") as psA,
            tc.tile_pool(name="psS", bufs=2, space="PSUM") as psS,
            tc.tile_pool(name="psV", bufs=4, space="PSUM") as psV,
        ):
            # ---- constants / weights (persist) ----
            wq_sb = []
            for c in range(8):
                t = pp.tile([128, FQKV], F32R, name=f"wq{c}", tag=f"wq{c}")
                nc.sync.dma_start(t[:], wq_d[c * 128:(c + 1) * 128, :])
                wq_sb.append(t)
            wp_sb = []
            for c in range(8):
                t = pp.tile([128, C], BF16, name=f"wp{c}", tag=f"wp{c}")
                nc.gpsimd.dma_start(t[:], wp_d[c * 128:(c + 1) * 128, :])
                wp_sb.append(t)
            perm_sb = pp.tile([128, 128], F32R, name="perm_sb", tag="perm_sb")
            nc.gpsimd.dma_start(perm_sb[:], perm_d[:])
            id_sb = pp.tile([128, 128], F32, name="id_sb", tag="id_sb")
            nc.gpsimd.dma_start(id_sb[:], id_d[:])
            mask_sb = []
            for m in range(4):
                t = pp.tile([128, 512], F32R, name=f"mask{m}", tag=f"mask{m}")
                nc.gpsimd.dma_start(t[:], mask_d[m])
                mask_sb.append(t)
            ones_c = pp.tile([128, 1], F32, name="ones_c", tag="ones_c")
            nc.vector.memset(ones_c[:], 1.0)
            ones_r = pp.tile([1, HD], F32R, name="ones_r", tag="ones_r")
            nc.vector.tensor_copy(ones_r[:], ones_c[0:1, 0:1].broadcast_to((1, HD)))

            # chunked activations: 8 chunks of [128, 512] each
            qtc = [pp.tile([128, 512], F32R, name=f"qtc{i}", tag=f"qtc{i}")
                   for i in range(NCH)]
            ktc = [pp.tile([128, 512], F32R, name=f"ktc{i}", tag=f"ktc{i}")
                   for i in range(NCH)]
            vtc = [pp.tile([128, 512], F32, name=f"vtc{i}", tag=f"vtc{i}")
                   for i in range(NCH)]
            fdst = [qtc, ktc, vtc]

            # ---- phase 1: qkvT = wqkvT.T @ xT, [f, t] layout ----
            for th in range(4):          # t quarters to bound xT residency
                xt_sb = []
                for c in range(8):
                    t = pp.tile([128, 1024], F32R, name=f"xt{th}{c}",
                                tag=f"xt{c}")
                    nc.sync.dma_start(t[:], xT_d[c * 128:(c + 1) * 128,
                                                 th * 1024:(th + 1) * 1024])
                    xt_sb.append(t)
                for f in range(3):
                    pq = [psA.tile([128, 512], F32, name=f"pq{th}{f}{tq}",
                                   tag="ps_a") for tq in range(2)]
                    for c in range(8):
                        for tq in range(2):
                            nc.tensor.matmul(
                                pq[tq][:],
                                wq_sb[c][:, f * 128:(f + 1) * 128],
                                xt_sb[c][:, tq * 512:(tq + 1) * 512],
                                start=(c == 0), stop=(c == 7),
                            )
                    for tq in range(2):
                        nc.scalar.copy(fdst[f][th * 2 + tq][:], pq[tq][:])

            # ---- phase 2: RoPE on q, k chunks (in place) ----
            for ch in range(NCH):
                cosc = wk.tile([128, 512], F32R, name=f"cosc{ch}", tag="cosc")
                nc.gpsimd.dma_start(cosc[:], cos_d[:, ch * 512:(ch + 1) * 512])
                sinc = wk.tile([128, 512], F32R, name=f"sinc{ch}", tag="sinc")
                nc.gpsimd.dma_start(sinc[:], sin_d[:, ch * 512:(ch + 1) * 512])
                for which, tcl in (("q", qtc), ("k", ktc)):
                    src = tcl[ch]
                    pr = psA.tile([128, 512], F32, name=f"pr{which}{ch}",
                                  tag="ps_a")
                    nc.tensor.matmul(pr[:], perm_sb[:], src[:],
                                     start=True, stop=True)
                    rot = wk.tile([128, 512], F32R, name=f"rot{which}{ch}",
                                  tag="rot")
                    nc.vector.tensor_mul(rot[:], pr[:], sinc[:])
                    nc.vector.tensor_mul(src[:], src[:], cosc[:])
                    nc.vector.tensor_add(src[:], src[:], rot[:])

            # ---- phase 3: V blocks [t, d] with ones columns ----
            v_sb = []
            for kb in range(BT // 128):  # 32 key blocks across both batches
                pv = psA.tile([128, 128], F32, name=f"pv{kb}", tag="ps_a")
                nc.tensor.transpose(
                    pv[:], vtc[kb // 4][:, (kb % 4) * 128:(kb % 4 + 1) * 128],
                    id_sb[:])
                v = pp.tile([128, 2 * (HD + 1)], F32R, name=f"v{kb}",
                            tag=f"v{kb}")
                nc.vector.tensor_copy(v[:, 0:HD], pv[:, 0:HD])
                nc.vector.tensor_copy(v[:, HD + 1:2 * HD + 1],
                                      pv[:, HD:2 * HD])
                nc.vector.tensor_copy(v[:, HD:HD + 1], ones_c[:])
                nc.vector.tensor_copy(v[:, 2 * HD + 1:2 * HD + 2], ones_c[:])
                v_sb.append(v)

            # ---- phase 4: attention per (batch, head), kb-outer ----
            def normalize(b, h, qc, avq):
                den = wk.tile([HD + 1, 512], F32, name=f"den{b}{h}{qc}",
                              tag="den")
                nc.scalar.copy(den[HD:HD + 1, :], avq[HD:HD + 1, :])
                nc.sync.dma_start(den_dram[b, h, qc][:], den[HD:HD + 1, :])
                bcd = wk.tile([HD, 512], F32, name=f"bcd{b}{h}{qc}",
                              tag="bcd")
                nc.sync.dma_start(
                    bcd[:], den_dram[b, h, qc][0:1, :].broadcast_to((HD, 512)))
                bc = wk.tile([HD, 512], F32, name=f"bc{b}{h}{qc}", tag="bc")
                scr = wk.tile([HD, 512], F32, name=f"scr{b}{h}{qc}",
                              tag="scr")
                nc.vector.reciprocal_approx_accurate(
                    out=bc[:], in_=bcd[:], scratch=scr[:])
                st = wk.tile([HD, 512], BF16, name=f"st{b}{h}{qc}", tag="st")
                nc.vector.tensor_mul(st[:], avq[0:HD, :], bc[:])
                # stage into A2A blocks 2qc (first 256 tokens) / 2qc+1
                for half in range(2):
                    r0 = 128 * (2 * qc + half) + HD * h
                    nc.sync.dma_start(
                        a2a_in[b][r0:r0 + HD, :],
                        st[:, half * 256:(half + 1) * 256])

            def attention(b, on_qc_done):
                for h in range(HL):
                    hp = h * 64
                    avp = [psV.tile([HD + 1, 512], F32, name=f"av{b}{h}{q_}",
                                    tag="ps_av") for q_ in range(4)]

                    def s_group(kb, b=b, h=h, hp=hp):
                        """S^T + exp (+mask) for all valid q chunks of kb."""
                        res = []
                        kch = ktc[b * 4 + kb // 4]
                        koff = (kb % 4) * 128
                        for qc in range(kb // 4, 4):
                            sps = psS.tile([128, 512], F32,
                                           name=f"s{b}{h}{kb}{qc}",
                                           tag="ps_s")
                            nc.tensor.matmul(
                                sps[:],
                                kch[hp:hp + 64, koff:koff + 128],
                                qtc[b * 4 + qc][hp:hp + 64, :],
                                start=True, stop=True,
                            )
                            pt = ptp.tile([128, 512], F32R,
                                          name=f"pt{b}{h}{kb}{qc}", tag="pt")
                            nc.scalar.activation(
                                pt[:], sps[:],
                                mybir.ActivationFunctionType.Exp,
                                scale=0.125,
                            )
                            if qc == kb // 4:
                                nc.vector.tensor_mul(
                                    pt[:], pt[:], mask_sb[kb % 4][:])
                            res.append((qc, pt))
                        return res

                    def av_group(kb, pts, b=b, h=h, avp=avp):
                        for qc, pt in pts:
                            nc.tensor.matmul(
                                avp[qc][:],
                                v_sb[b * 16 + kb][:, h * (HD + 1):
                                                  (h + 1) * (HD + 1)],
                                pt[:],
                                start=(kb == 0), stop=(kb == 4 * qc + 3),
                            )

                    # software-pipeline S one kb-group ahead of AV; emit the
                    # normalize chain for q chunk qc right after its AV stop
                    # (kb == 4qc+3) so downstream engines start early.
                    prev = s_group(0)
                    for kb in range(1, 16):
                        cur = s_group(kb)
                        av_group(kb - 1, prev)
                        if (kb - 1) % 4 == 3:
                            qc = (kb - 1) // 4
                            normalize(b, h, qc, avp[qc])
                            if h == 1:
                                on_qc_done(b, qc)
                        prev = cur
                    av_group(15, prev)
                    normalize(b, h, 3, avp[3])
                    if h == 1:
                        on_qc_done(b, 3)

            # ---- phase 5: AllToAll + token-sharded out-projection ----
            def a2a_issue(b):
                nc.gpsimd.collective_compute(
                    "AllToAll",
                    mybir.AluOpType.bypass,
                    replica_groups=[list(range(NC))],
                    ins=[a2a_in[b][:]],
                    outs=[a2a_out[b][:]],
                )

            def proj(b):
                """Full out-proj for this core's 256 tokens of batch b."""
                pl = []
                for c in range(8):
                    t = wk.tile([128, 256], BF16, name=f"pl{b}{c}",
                                tag=f"pl{c}")
                    nc.sync.dma_start(
                        t[:], a2a_out[b][c * 128:(c + 1) * 128, :])
                    pl.append(t)
                for tb in range(2):
                    for oh in range(2):
                        po = psA.tile([128, 512], F32,
                                      name=f"po{b}{tb}{oh}", tag="ps_a")
                        for c in range(8):
                            nc.tensor.matmul(
                                po[:],
                                pl[c][:, tb * 128:(tb + 1) * 128],
                                wp_sb[c][:, oh * 512:(oh + 1) * 512],
                                start=(c == 0), stop=(c == 7),
                            )
                        pf = wk.tile([128, 512], F32,
                                     name=f"pf{b}{tb}{oh}", tag="pf")
                        nc.vector.tensor_copy(pf[:], po[:])
                        nc.sync.dma_start(
                            out_d[b * 256 + tb * 128:b * 256 + (tb + 1) * 128,
                                  oh * 512:(oh + 1) * 512], pf[:])

            def noop(b, qc):
                pass

            attention(0, noop)
            a2a_issue(0)
            attention(1, noop)
            a2a_issue(1)
            proj(0)
            proj(1)

    nc.finalize()
    return nc


def host_inputs(x, w_attn, w_proj):
    """Host-side sharding/layout prep. Returns per-core in_maps."""
    x2 = np.ascontiguousarray(x.reshape(BT, C).T).astype(np.float32)  # [C,BT]

    inv = 1.0 / (ROPE_BASE ** (np.arange(0, HD, 2, dtype=np.float32) / HD))
    tpos = np.arange(T, dtype=np.float32)
    freqs = tpos[:, None] * inv[None, :]                  # [T, 32]
    emb = np.concatenate([freqs, freqs], axis=-1)         # [T, 64]
    cosT = np.cos(emb).T.astype(np.float32)               # [64, T]
    sinT = np.sin(emb).T.astype(np.float32)
    cos_full = np.ascontiguousarray(np.tile(cosT, (2, B)))  # [128, BT]
    sin_full = np.ascontiguousarray(np.tile(sinT, (2, B)))

    m64 = np.zeros((HD, HD), dtype=np.float32)
    half = HD // 2
    for d in range(half):
        m64[d, d + half] = -1.0
        m64[d + half, d] = 1.0
    perm = np.zeros((128, 128), dtype=np.float32)
    perm[0:HD, 0:HD] = m64
    perm[HD:128, HD:128] = m64
    permT = np.ascontiguousarray(perm.T)

    masks = np.zeros((4, 128, 512), dtype=np.float32)
    qi = np.arange(512)[None, :]
    ki = np.arange(128)[:, None]
    for m in range(4):
        masks[m] = (qi - ki >= m * 128).astype(np.float32)

    ident = np.eye(128, dtype=np.float32)

    in_maps = []
    for i in range(NC):
        r0 = i * (HL * HD)
        wq = w_attn[r0:r0 + HL * HD, :]
        wk_ = w_attn[C + r0:C + r0 + HL * HD, :]
        wv = w_attn[2 * C + r0:2 * C + r0 + HL * HD, :]
        wqkvT = np.ascontiguousarray(
            np.concatenate([wq, wk_, wv], axis=0).T).astype(np.float32)
        wpT = np.ascontiguousarray(w_proj.T).astype(ml_dtypes.bfloat16)
        in_maps.append({
            "xT": x2, "wqkvT": wqkvT, "wpT": wpT,
            "cosT": cos_full, "sinT": sin_full, "permT": permT,
            "masks": masks, "ident": ident,
        })
    return in_maps


_NC_CACHE = None


def _get_nc():
    global _NC_CACHE
    if _NC_CACHE is None:
        _NC_CACHE = build()
    return _NC_CACHE


def run(x, w_attn, w_proj, trace=False):
    nc = _get_nc()
    in_maps = host_inputs(np.asarray(x), np.asarray(w_attn),
                          np.asarray(w_proj))
    res = run_bass_kernel_spmd(nc, in_maps, list(range(NC)), trace=trace)
    # core i returns [512, 1024]: rows 0:256 = batch0 rows [256i, 256i+256),
    # rows 256:512 = batch1 rows [256i, 256i+256)
    out = np.empty((B, T, C), dtype=np.float32)
    piece = T // NC
    for i in range(NC):
        sh = res.results[i]["out"]
        out[0, i * piece:(i + 1) * piece] = sh[0:piece]
        out[1, i * piece:(i + 1) * piece] = sh[piece:2 * piece]
    return out, res


def kernel(x, w_attn, w_proj):
    out, _ = run(x, w_attn, w_proj, trace=False)
    return out



# revision 32
# speedup vs baseline: 1.8201x; 1.4651x over previous
"""Causal self-attention with RoPE on 8 TRN2 NeuronCores.

Head-parallel tensor parallelism: core i owns heads 2i, 2i+1. Each core
computes its slice of the qkv projection (bf16 operands, fp32 psum),
per-head causal attention in SBUF with diagonal-block slicing, then the
normalized attention outputs are exchanged with a per-(batch, head)
AllToAll so every core performs the full output projection for its own
512-token shard.

Scheduling notes:
- all matmuls are bf16 (full PE rate; fp32r is no faster and costs 2x
  DMA/SBUF), PSUM always fp32.
- S blocks for one query chunk are emitted in groups >= 4 per shape to
  amortize the PE tile-geometry switch (~100 ns when alternating).
- exp() runs on the Scalar engine only (the only engine with exp); it is
  the attention-phase co-bottleneck, so Scalar does nothing else and S
  psum tiles pair two key-blocks so one activation covers [128, 1024].
- the qkv/rope/v-transpose work for batch 1 and the batch-0 projection
  are emitted as filler inside the (Scalar-bound) attention windows of
  batch 0 / batch 1 to keep the PE p-state ramped.
"""

import ml_dtypes
import numpy as np

import concourse.bass as bass
import concourse.mybir as mybir
import concourse.tile as tile
from concourse import bacc
from concourse.bass_utils import run_bass_kernel_spmd

F32 = mybir.dt.float32
BF16 = mybir.dt.bfloat16

B, T, C = 2, 2048, 1024
H, HD = 16, 64
NC = 8
HL = H // NC          # heads per core = 2
BT = B * T            # 4096
FQKV = 3 * HL * HD    # 384 rows of w_attn per core
TSH = BT // NC        # 512 output rows per core
NCH = BT // 512       # 8 column chunks of the [*, BT] activations
ROPE_BASE = 10000.0


DEBUG = False


def build():
    nc = bacc.Bacc(None, target_bir_lowering=False)

    xT_d = nc.dram_tensor("xT", [C, BT], BF16, kind="ExternalInput")
    wq_d = nc.dram_tensor("wqkvT", [C, FQKV], BF16, kind="ExternalInput")
    wp_d = nc.dram_tensor("wpT", [C, C], BF16, kind="ExternalInput")
    cos_d = nc.dram_tensor("cosT", [128, BT], BF16, kind="ExternalInput")
    sin_d = nc.dram_tensor("sinT", [128, BT], BF16, kind="ExternalInput")
    perm_d = nc.dram_tensor("permT", [128, 128], BF16, kind="ExternalInput")
    tri_d = nc.dram_tensor("tri", [128, 128], BF16, kind="ExternalInput")
    id_d = nc.dram_tensor("identF", [128, 128], F32, kind="ExternalInput")
    out_d = nc.dram_tensor("out", [TSH, C], F32, kind="ExternalOutput")
    dbg = {}
    if DEBUG:
        dbg['q0'] = nc.dram_tensor("dbg_q0", [128, 512], BF16,
                                   kind="ExternalOutput")
        dbg['k0'] = nc.dram_tensor("dbg_k0", [128, 512], BF16,
                                   kind="ExternalOutput")
        dbg['v0'] = nc.dram_tensor("dbg_v0", [128, 512], F32,
                                   kind="ExternalOutput")
        dbg['vsb0'] = nc.dram_tensor("dbg_vsb0", [128, 130], BF16,
                                     kind="ExternalOutput")
        dbg['pt0'] = nc.dram_tensor("dbg_pt0", [128, 1024], BF16,
                                    kind="ExternalOutput")
        dbg['av0'] = nc.dram_tensor("dbg_av0", [65, 512], F32,
                                    kind="ExternalOutput")
        dbg['st0'] = nc.dram_tensor("dbg_st0", [64, 512], BF16,
                                    kind="ExternalOutput")
        dbg['rc0'] = nc.dram_tensor("dbg_rc0", [1, 512], F32,
                                    kind="ExternalOutput")
        dbg['bcs0'] = nc.dram_tensor("dbg_bcs0", [64, 512], F32,
                                     kind="ExternalOutput")
        dbg['a2a0'] = nc.dram_tensor("dbg_a2a0", [512, 256], BF16,
                                     kind="ExternalOutput")

    # AllToAll staging per (batch, head): block j (rows 64j:64j+64) = this
    # core's 64 head-h channels for core j's 256 tokens of batch b.
    a2a_in = {(b_, h_): nc.dram_tensor(f"a2ain{b_}{h_}", [NC * HD, T // NC],
                                       BF16)
              for b_ in range(2) for h_ in range(2)}
    a2a_out = {(b_, h_): nc.dram_tensor(f"a2aout{b_}{h_}", [NC * HD, T // NC],
                                        BF16)
               for b_ in range(2) for h_ in range(2)}

    with tile.TileContext(nc) as tc:
        with (
            tc.tile_pool(name="persist", bufs=1) as pp,
            tc.tile_pool(name="work", bufs=2) as wk,
            tc.tile_pool(name="xtp", bufs=2) as xtp,
            tc.tile_pool(name="ptp", bufs=1) as ptp,
            tc.tile_pool(name="ps", bufs=1, space="PSUM") as ps,
        ):
            # ---------- constants / weights ----------
            wq_sb = []
            for c in range(8):
                t = pp.tile([128, FQKV], BF16, name=f"wq{c}", tag=f"wq{c}")
                nc.gpsimd.dma_start(t[:], wq_d[c * 128:(c + 1) * 128, :])
                wq_sb.append(t)
            perm_sb = pp.tile([128, 128], BF16, name="perm_sb", tag="perm_sb")
            nc.gpsimd.dma_start(perm_sb[:], perm_d[:])
            id_sb = pp.tile([128, 128], F32, name="id_sb", tag="id_sb")
            nc.gpsimd.dma_start(id_sb[:], id_d[:])
            tri_sb = pp.tile([128, 128], BF16, name="tri_sb", tag="tri_sb")
            nc.gpsimd.dma_start(tri_sb[:], tri_d[:])
            cos_sb = pp.tile([128, BT], BF16, name="cos_sb", tag="cos_sb")
            nc.gpsimd.dma_start(cos_sb[:], cos_d[:])
            sin_sb = pp.tile([128, BT], BF16, name="sin_sb", tag="sin_sb")
            nc.gpsimd.dma_start(sin_sb[:], sin_d[:])
            wp_sb = []
            for c in range(8):
                t = pp.tile([128, C], BF16, name=f"wp{c}", tag=f"wp{c}")
                nc.gpsimd.dma_start(t[:], wp_d[c * 128:(c + 1) * 128, :])
                wp_sb.append(t)

            onesf = pp.tile([128, 64], F32, name="onesf", tag="onesf")
            nc.vector.memset(onesf[:], 1.0)
            ones_c = pp.tile([128, 1], BF16, name="ones_c", tag="ones_c")
            nc.vector.tensor_copy(ones_c[:], onesf[:, 0:1])
            # all-ones [65,64]; row 64 is the partition-64-aligned stationary
            # for the denominator-broadcast matmul
            ones65 = pp.tile([65, 64], BF16, name="ones65", tag="ones65")
            nc.vector.tensor_copy(ones65[:], onesf[0:65, :])

            # ---------- PSUM slots ----------
            psS = [ps.tile([128, 1024], F32, name=f"psS{i}", tag=f"psS{i}")
                   for i in range(2)]
            av = [ps.tile([65, 512], F32, name=f"av{i}", tag=f"av{i}")
                  for i in range(2)]
            bcp = ps.tile([128, 512], F32, name="bcp", tag="bcp")
            pqp = ps.tile([128, 512], F32, name="pqp", tag="pqp")

            def ps_slot(i):
                """Six [128,512] qkv psum regions for one t-quarter."""
                if i < 4:
                    return psS[i // 2][:, (i % 2) * 512:(i % 2 + 1) * 512]
                return (bcp if i == 4 else pqp)[:]

            # ---------- activation chunks ----------
            qtc = [pp.tile([128, 512], BF16, name=f"qtc{i}", tag=f"qtc{i}")
                   for i in range(NCH)]
            ktc = [pp.tile([128, 512], BF16, name=f"ktc{i}", tag=f"ktc{i}")
                   for i in range(NCH)]
            vtc = [pp.tile([128, 512], F32, name=f"vtc{i}", tag=f"vtc{i}")
                   for i in range(NCH)]
            fdst = [qtc, ktc, vtc]
            v_sb = [pp.tile([128, 130], BF16, name=f"v{kb}", tag=f"v{kb}")
                    for kb in range(BT // 128)]

            # ---------- phase 1 pieces (also used as attention filler) ----
            def load_xt(th):
                xt = []
                for c in range(8):
                    t = xtp.tile([128, 1024], BF16, name=f"xt{th}{c}",
                                 tag=f"xt{c}")
                    nc.sync.dma_start(t[:], xT_d[c * 128:(c + 1) * 128,
                                                 th * 1024:(th + 1) * 1024])
                    xt.append(t)
                return xt

            def qkv_quantum(th, xt, f, tq, slot=None):
                # psum->sbuf copy: Scalar pre-attention (idle then), DVE
                # when running as filler inside the exp-bound windows
                eng = nc.scalar if slot is None else nc.vector
                if slot is None:
                    slot = ps_slot(f * 2 + tq)
                for c in range(8):
                    nc.tensor.matmul(
                        slot,
                        wq_sb[c][:, f * 128:(f + 1) * 128],
                        xt[c][:, tq * 512:(tq + 1) * 512],
                        start=(c == 0), stop=(c == 7),
                    )
                ch = th * 2 + tq
                if eng is nc.scalar:
                    eng.copy(fdst[f][ch][:], slot)
                else:
                    eng.tensor_copy(fdst[f][ch][:], slot)

            def rope_quantum(ch):
                """RoPE in place on q and k chunk ch."""
                for which, tcl in ((0, qtc), (1, ktc)):
                    nc.tensor.matmul(pqp[:], perm_sb[:], tcl[ch][:],
                                     start=True, stop=True)
                    rot = wk.tile([128, 512], BF16, name=f"rot{which}{ch}",
                                  tag="rot")
                    nc.vector.tensor_mul(
                        rot[:], pqp[:], sin_sb[:, ch * 512:(ch + 1) * 512])
                    nc.vector.tensor_mul(
                        tcl[ch][:], tcl[ch][:],
                        cos_sb[:, ch * 512:(ch + 1) * 512])
                    nc.vector.tensor_add(tcl[ch][:], tcl[ch][:], rot[:])

            def vt_quantum(ch):
                """Transpose v chunk ch into 4 v_sb key blocks."""
                for j in range(4):
                    kb = ch * 4 + j
                    pv = bcp[:, 0:128] if j % 2 == 0 else bcp[:, 128:256]
                    nc.tensor.transpose(pv,
                                        vtc[ch][:, j * 128:(j + 1) * 128],
                                        id_sb[:])
                    v = v_sb[kb]
                    nc.vector.tensor_copy(v[:, 0:64], pv[:, 0:64])
                    nc.vector.tensor_copy(v[:, 65:129], pv[:, 64:128])
                    nc.vector.tensor_copy(v[:, 64:65], ones_c[:])
                    nc.vector.tensor_copy(v[:, 129:130], ones_c[:])

            # ---------- attention ----------
            pt_tiles = [ptp.tile([128, 1024], BF16, name=f"pt{i}",
                                 tag=f"pt{i}") for i in range(6)]
            pt_idx = [0]

            def attn_head(b, h, filler):
                hp = h * 64
                pending_norm = [None]

                def s_pair(pi, qc, kb_a, kb_b):
                    """S + exp for key blocks kb_a, kb_b of query chunk qc.
                    kb_b may be None. Returns (pt, [(kb, off, F), ...])."""
                    sp = psS[pi % 2]
                    pt = pt_tiles[pt_idx[0] % 6]
                    pt_idx[0] += 1
                    ent = []
                    for slot_i, kb in enumerate((kb_a, kb_b)):
                        if kb is None:
                            continue
                        koff = max(0, (kb - 4 * qc) * 128)
                        F = 512 - koff
                        off = slot_i * 512
                        kch = ktc[b * 4 + kb // 4]
                        nc.tensor.matmul(
                            sp[:, off:off + F],
                            kch[hp:hp + 64, (kb % 4) * 128:(kb % 4 + 1) * 128],
                            qtc[b * 4 + qc][hp:hp + 64, koff:512],
                            start=True, stop=True,
                        )
                        ent.append((kb, off, F, koff))
                    diag = ent[0][3] > 0 or (len(ent) > 1 and ent[1][3] > 0) \
                        or any(kb >= 4 * qc for kb, _, _, _ in ent)
                    if not diag:
                        nc.scalar.activation(
                            pt[:, 0:1024], sp[:, 0:1024],
                            mybir.ActivationFunctionType.Exp, scale=0.125)
                    else:
                        for kb, off, F, koff in ent:
                            nc.scalar.activation(
                                pt[:, off:off + F], sp[:, off:off + F],
                                mybir.ActivationFunctionType.Exp, scale=0.125)
                            if kb >= 4 * qc:
                                nc.vector.tensor_mul(
                                    pt[:, off:off + 128], pt[:, off:off + 128],
                                    tri_sb[:])
                    if DEBUG and b == 0 and h == 0 and qc == 0 and pi == 0:
                        nc.sync.dma_start(dbg['pt0'][:], pt[:])
                    return (pt, ent)

                def av_pair(avq, qc, pair, nkb):
                    pt, ent = pair
                    for kb, off, F, koff in ent:
                        nc.tensor.matmul(
                            avq[:, koff:512],
                            v_sb[b * 16 + kb][:, h * 65:(h + 1) * 65],
                            pt[:, off:off + F],
                            start=(kb == 0), stop=(kb == nkb - 1),
                        )

                for qc in range(4):
                    nkb = 4 * qc + 4
                    avq = av[qc % 2]
                    pairs = []
                    for p in range(0, nkb, 2):
                        pairs.append((p // 2, qc, p,
                                      p + 1 if p + 1 < nkb else None))
                    # emit S pairs and AV pairs in groups of 2 pairs (4 kb),
                    # AV lagging one group behind S
                    done = []
                    groups = [pairs[g:g + 2] for g in range(0, len(pairs), 2)]
                    prev = None
                    for gi, grp in enumerate(groups):
                        cur = [s_pair(*args) for args in grp]
                        if pending_norm[0] is not None:
                            pending_norm[0]()
                            pending_norm[0] = None
                        if prev is not None:
                            for pr_ in prev:
                                av_pair(avq, qc, pr_, nkb)
                        if filler:
                            filler(qc)
                        prev = cur
                    for pr_ in prev:
                        av_pair(avq, qc, pr_, nkb)

                    def norm(b=b, h=h, qc=qc, avq=avq):
                        if DEBUG and b == 0 and h == 0 and qc == 0:
                            avf = wk.tile([65, 512], F32, name="avf",
                                          tag="avf")
                            nc.vector.tensor_copy(avf[:], avq[:])
                            nc.sync.dma_start(dbg['av0'][:], avf[:])
                        # den (psum row 64) -> sbuf bf16 on its own
                        # partition, PE-broadcast to partitions 0:64, then
                        # reciprocal on the [64,512] base-0 region
                        dsb = wk.tile([65, 512], BF16, name=f"ds{b}{h}{qc}",
                                      tag="dsb")
                        nc.scalar.copy(dsb[64:65, :], avq[64:65, :])
                        nc.tensor.matmul(bcp[0:64, :], ones65[64:65, :],
                                         dsb[64:65, :],
                                         start=True, stop=True)
                        bcs = wk.tile([64, 512], F32, name=f"bcs{b}{h}{qc}",
                                      tag="bcs")
                        scr = wk.tile([64, 512], F32, name=f"scr{b}{h}{qc}",
                                      tag="scr")
                        nc.vector.reciprocal_approx_accurate(
                            out=bcs[:], in_=bcp[0:64, :], scratch=scr[:])
                        if DEBUG and b == 0 and h == 0 and qc == 0:
                            nc.sync.dma_start(dbg['rc0'][:], bcs[0:1, :])
                            nc.sync.dma_start(dbg['bcs0'][:], bcs[:])
                        st = wk.tile([64, 512], BF16, name=f"st{b}{h}{qc}",
                                     tag="st")
                        nc.vector.tensor_mul(st[:], avq[0:64, :], bcs[:])
                        if DEBUG and b == 0 and h == 0 and qc == 0:
                            nc.sync.dma_start(dbg['st0'][:], st[:])
                        for half in range(2):
                            r0 = 64 * (2 * qc + half)
                            nc.sync.dma_start(
                                a2a_in[b, h][r0:r0 + 64, :],
                                st[:, half * 256:(half + 1) * 256])
                    pending_norm[0] = norm
                pending_norm[0]()

            def a2a_issue(b, h):
                nc.gpsimd.collective_compute(
                    "AllToAll",
                    mybir.AluOpType.bypass,
                    replica_groups=[list(range(NC))],
                    ins=[a2a_in[b, h][:]],
                    outs=[a2a_out[b, h][:]],
                )

            # ---------- out-projection for this core's tokens ----------
            def proj_load(b):
                pl = []
                for c in range(8):
                    t = wk.tile([128, 256], BF16, name=f"pl{b}{c}",
                                tag=f"pl{c}")
                    for h in range(2):
                        nc.sync.dma_start(
                            t[h * 64:(h + 1) * 64, :],
                            a2a_out[b, h][c * 64:(c + 1) * 64, :])
                    pl.append(t)
                return pl

            def proj_quantum(b, pl, tb, oh):
                for c in range(8):
                    nc.tensor.matmul(
                        pqp[:],
                        pl[c][:, tb * 128:(tb + 1) * 128],
                        wp_sb[c][:, oh * 512:(oh + 1) * 512],
                        start=(c == 0), stop=(c == 7),
                    )
                pf = wk.tile([128, 512], F32, name=f"pf{b}{tb}{oh}",
                             tag="pf")
                nc.vector.tensor_copy(pf[:], pqp[:])
                nc.sync.dma_start(
                    out_d[b * 256 + tb * 128:b * 256 + (tb + 1) * 128,
                          oh * 512:(oh + 1) * 512], pf[:])

            # ================= schedule =================
            # phase 1 for batch 0 (th 0, 1)
            xt0 = load_xt(0)
            xt1 = load_xt(1)
            for f in range(3):
                for tq in range(2):
                    qkv_quantum(0, xt0, f, tq)
            for f in range(3):
                for tq in range(2):
                    qkv_quantum(1, xt1, f, tq)
            for ch in range(4):
                rope_quantum(ch)
                vt_quantum(ch)
            if DEBUG:
                nc.sync.dma_start(dbg['q0'][:], qtc[0][:])
                nc.sync.dma_start(dbg['k0'][:], ktc[0][:])
                nc.sync.dma_start(dbg['v0'][:], vtc[0][:])
                nc.sync.dma_start(dbg['vsb0'][:], v_sb[0][:])

            # filler generator: batch-1 phase 1 then batch-0 projection
            fill_q = []
            xt2 = load_xt(2)
            for f in range(3):
                for tq in range(2):
                    fill_q.append(
                        lambda f=f, tq=tq: qkv_quantum(2, xt2, f, tq, pqp[:]))
            fill_q.append(lambda: rope_quantum(4))
            fill_q.append(lambda: vt_quantum(4))
            fill_q.append(lambda: rope_quantum(5))
            fill_q.append(lambda: vt_quantum(5))

            fill_q2 = []
            xt3 = [None]

            def load3():
                xt3[0] = load_xt(3)
            fill_q2.append(load3)
            for f in range(3):
                for tq in range(2):
                    fill_q2.append(
                        lambda f=f, tq=tq: qkv_quantum(3, xt3[0], f, tq,
                                                       pqp[:]))
            fill_q2.append(lambda: rope_quantum(6))
            fill_q2.append(lambda: vt_quantum(6))
            fill_q2.append(lambda: rope_quantum(7))
            fill_q2.append(lambda: vt_quantum(7))

            def mk_filler(queue, per_point):
                def filler(qc):
                    for _ in range(per_point):
                        if queue:
                            queue.pop(0)()
                return filler

            def drain(queue):
                while queue:
                    queue.pop(0)()

            # batch 0 attention; fill with batch-1 phase-1 work
            fill_q.extend(fill_q2)
            attn_head(0, 0, mk_filler(fill_q, 2))
            a2a_issue(0, 0)
            attn_head(0, 1, mk_filler(fill_q, 2))
            a2a_issue(0, 1)
            drain(fill_q)

            # batch 1 attention; fill with batch-0 projection
            pl0 = [None]
            fill_p = []

            def loadp():
                pl0[0] = proj_load(0)
            fill_p.append(loadp)
            for tb in range(2):
                for oh in range(2):
                    fill_p.append(
                        lambda tb=tb, oh=oh: proj_quantum(0, pl0[0], tb, oh))

            attn_head(1, 0, mk_filler(fill_p, 1))
            a2a_issue(1, 0)
            attn_head(1, 1, mk_filler(fill_p, 1))
            a2a_issue(1, 1)
            drain(fill_p)

            pl1 = proj_load(1)
            for tb in range(2):
                for oh in range(2):
                    proj_quantum(1, pl1, tb, oh)
            if DEBUG:
                nc.sync.dma_start(dbg['a2a0'][:], a2a_out[0, 0][:])

    nc.finalize()
    return nc


def host_inputs(x, w_attn, w_proj):
    """Host-side sharding/layout prep. Returns per-core in_maps."""
    bf = ml_dtypes.bfloat16
    x2 = np.ascontiguousarray(x.reshape(BT, C).T).astype(bf)  # [C,BT]

    inv = 1.0 / (ROPE_BASE ** (np.arange(0, HD, 2, dtype=np.float32) / HD))
    tpos = np.arange(T, dtype=np.float32)
    freqs = tpos[:, None] * inv[None, :]                  # [T, 32]
    emb = np.concatenate([freqs, freqs], axis=-1)         # [T, 64]
    cosT = np.cos(emb).T.astype(np.float32)               # [64, T]
    sinT = np.sin(emb).T.astype(np.float32)
    cos_full = np.ascontiguousarray(np.tile(cosT, (2, B))).astype(bf)
    sin_full = np.ascontiguousarray(np.tile(sinT, (2, B))).astype(bf)

    m64 = np.zeros((HD, HD), dtype=np.float32)
    half = HD // 2
    for d in range(half):
        m64[d, d + half] = -1.0
        m64[d + half, d] = 1.0
    perm = np.zeros((128, 128), dtype=np.float32)
    perm[0:HD, 0:HD] = m64
    perm[HD:128, HD:128] = m64
    permT = np.ascontiguousarray(perm.T).astype(bf)

    qi = np.arange(128)[None, :]
    ki = np.arange(128)[:, None]
    tri = np.ascontiguousarray((qi >= ki).astype(np.float32)).astype(bf)

    identF = np.eye(128, dtype=np.float32)
    wpT = np.ascontiguousarray(w_proj.T).astype(bf)

    in_maps = []
    for i in range(NC):
        r0 = i * (HL * HD)
        wq = w_attn[r0:r0 + HL * HD, :]
        wk_ = w_attn[C + r0:C + r0 + HL * HD, :]
        wv = w_attn[2 * C + r0:2 * C + r0 + HL * HD, :]
        wqkvT = np.ascontiguousarray(
            np.concatenate([wq, wk_, wv], axis=0).T).astype(bf)
        in_maps.append({
            "xT": x2, "wqkvT": wqkvT, "wpT": wpT,
            "cosT": cos_full, "sinT": sin_full, "permT": permT,
            "tri": tri, "identF": identF,
        })
    return in_maps


_NC_CACHE = None


def _get_nc():
    global _NC_CACHE
    if _NC_CACHE is None:
        _NC_CACHE = build()
    return _NC_CACHE


def run(x, w_attn, w_proj, trace=False):
    nc = _get_nc()
    in_maps = host_inputs(np.asarray(x), np.asarray(w_attn),
                          np.asarray(w_proj))
    res = run_bass_kernel_spmd(nc, in_maps, list(range(NC)), trace=trace)
    # core i returns [512, 1024]: rows 0:256 = batch0 rows [256i, 256i+256),
    # rows 256:512 = batch1 rows [256i, 256i+256)
    out = np.empty((B, T, C), dtype=np.float32)
    piece = T // NC
    for i in range(NC):
        sh = res.results[i]["out"]
        out[0, i * piece:(i + 1) * piece] = sh[0:piece]
        out[1, i * piece:(i + 1) * piece] = sh[piece:2 * piece]
    return out, res


def kernel(x, w_attn, w_proj):
    out, _ = run(x, w_attn, w_proj, trace=False)
    return out
